# revision 12
# baseline (speedup 1.0000x reference)
"""Trainium2 Bass kernel for a single-step attention GRU decoder (AttnDecoderRNN).

Computation (see reference):
    embedded = emb[input]                                  (1, H)
    attn_w   = softmax(concat(e, h) @ attn_W.T + attn_b)   (1, L)
    attn_app = attn_w @ encoder_outputs                    (1, H)
    x        = relu(concat(e, attn_app) @ comb_W.T + comb_b)
    GRU step -> h_new                                      (1, H)
    logp     = log_softmax(h_new @ out_W.T + out_b)        (1, V)

Distribution over 8 NeuronCores:
  - out_W / out_b sharded over the (padded) vocab dim, bf16 weights.
  - GRU weights sharded over the gate-output dim (128 rows of each of r/z/n
    per core); h_new slices AllGathered.
  - attention / combine weights replicated (bf16); encoder outputs replicated.
  - log_softmax via per-core (max, sumexp) stats + tiny AllGather.

Host does only layout work: embedding-row gather, shard slicing, transposes,
dtype casts, padding, final unshard/concat.
"""

import os
import sys

for _p in ("/opt/trn_rl_repo",):
    if _p not in sys.path and os.path.isdir(_p):
        sys.path.insert(0, _p)

import numpy as np
import ml_dtypes

import concourse.bass as bass
import concourse.bacc as bacc
import concourse.tile as tile
import concourse.mybir as mybir
import concourse.bass_utils as bass_utils

BF16 = ml_dtypes.bfloat16
F32 = mybir.dt.float32
BF = mybir.dt.bfloat16
AX = mybir.AxisListType
ALU = mybir.AluOpType
ACTF = mybir.ActivationFunctionType

H = 1024
L = 512
V = 50257
NCORES = 8
KH = H // 128          # 8 h-chunks
VS = 6656              # per-core padded vocab shard (13 * 512)
VP = NCORES * VS       # 53248 padded vocab
NT = VS // 512         # 13 matvec tiles per core
RG = [list(range(NCORES))]

LAST_RESULT = None     # BassKernelResults of the most recent run (for test.py)

_CACHED = None


def _build():
    """Build + compile the SPMD graph (identical on all 8 cores)."""
    nc = bacc.Bacc("TRN2", target_bir_lowering=False, debug=False,
                   num_devices=NCORES)

    # ---- DRAM I/O -------------------------------------------------------
    ecol_in = nc.dram_tensor("ecol", [128, KH], BF, kind="ExternalInput")
    hcol_in = nc.dram_tensor("hcol", [128, KH], BF, kind="ExternalInput")
    hsl_in = nc.dram_tensor("hsl", [1, 128], F32, kind="ExternalInput")
    wa_in = nc.dram_tensor("wa", [128, 16 * L], BF, kind="ExternalInput")
    enc_in = nc.dram_tensor("enc", [128, 4 * H], BF, kind="ExternalInput")
    comb_in = nc.dram_tensor("comb", [128, 16 * H], BF, kind="ExternalInput")
    wih_in = nc.dram_tensor("wih", [128, KH * 384], BF, kind="ExternalInput")
    whh_in = nc.dram_tensor("whh", [128, KH * 384], BF, kind="ExternalInput")
    bih_in = nc.dram_tensor("bih", [1, 384], F32, kind="ExternalInput")
    bhh_in = nc.dram_tensor("bhh", [1, 384], F32, kind="ExternalInput")
    ab_in = nc.dram_tensor("ab", [1, L], F32, kind="ExternalInput")
    cb_in = nc.dram_tensor("cb", [1, H], F32, kind="ExternalInput")
    wout_in = nc.dram_tensor("wout", [NT, 128, KH * 512], BF, kind="ExternalInput")
    bout_in = nc.dram_tensor("bout", [1, VS], BF, kind="ExternalInput")

    out_logp = nc.dram_tensor("out_logp", [1, VS], F32, kind="ExternalOutput")
    out_h = nc.dram_tensor("out_h", [1, H], F32, kind="ExternalOutput")
    out_attn = nc.dram_tensor("out_attn", [1, L], F32, kind="ExternalOutput")

    with tile.TileContext(nc) as tc:
        with (
            tc.tile_pool(name="wpool", bufs=1) as wp,
            tc.tile_pool(name="wtpool", bufs=3) as wtp,
            tc.tile_pool(name="rows", bufs=1) as rp,
            tc.tile_pool(name="esc", bufs=2) as ep,
            tc.tile_pool(name="cols", bufs=1) as cp,
            tc.tile_pool(name="chain_ps", bufs=4, space="PSUM") as pp,
            tc.tile_pool(name="mv_ps", bufs=3, space="PSUM") as mp,
            tc.tile_pool(name="dram", bufs=1, space="DRAM") as dp,
        ):
            # ---- warm-up collective (absorbs ncfw init, runs concurrently)
            warm = rp.tile([1, 64], F32, tag="warm")
            nc.gpsimd.memset(warm[:], 0.0)
            win_b = dp.tile([1, 64], F32, tag="warm_in")
            wout_b = dp.tile([8, 64], F32, tag="warm_out")
            nc.gpsimd.dma_start(win_b[:], warm[:])
            nc.gpsimd.collective_compute(
                "AllGather", ALU.bypass, replica_groups=RG,
                ins=[win_b.opt()], outs=[wout_b.opt()])
            wrb = rp.tile([1, 1], F32, tag="warm_rb")
            nc.gpsimd.dma_start(wrb[:], wout_b[0:1, 0:1])

            # ---- weight / vector loads -----------------------------------
            wa_sb = wp.tile([128, 16 * L], BF, tag="wa")
            nc.sync.dma_start(wa_sb[:], wa_in[:])
            enc_sb = wp.tile([128, 4 * H], BF, tag="enc")
            nc.sync.dma_start(enc_sb[:], enc_in[:])
            comb_sb = wp.tile([128, 16 * H], BF, tag="comb")
            nc.sync.dma_start(comb_sb[:], comb_in[:])
            wih_sb = wp.tile([128, KH * 384], BF, tag="wih")
            nc.sync.dma_start(wih_sb[:], wih_in[:])
            whh_sb = wp.tile([128, KH * 384], BF, tag="whh")
            nc.sync.dma_start(whh_sb[:], whh_in[:])

            ecol = cp.tile([128, KH], BF, tag="ecol")
            nc.gpsimd.dma_start(ecol[:], ecol_in[:])
            hcol = cp.tile([128, KH], BF, tag="hcol")
            nc.gpsimd.dma_start(hcol[:], hcol_in[:])
            hsl = rp.tile([1, 128], F32, tag="hsl")
            nc.gpsimd.dma_start(hsl[:], hsl_in[:])
            ab_sb = rp.tile([1, L], F32, tag="ab")
            nc.gpsimd.dma_start(ab_sb[:], ab_in[:])
            cb_sb = rp.tile([1, H], F32, tag="cb")
            nc.gpsimd.dma_start(cb_sb[:], cb_in[:])
            bih_sb = rp.tile([1, 384], F32, tag="bih")
            nc.gpsimd.dma_start(bih_sb[:], bih_in[:])
            bhh_sb = rp.tile([1, 384], F32, tag="bhh")
            nc.gpsimd.dma_start(bhh_sb[:], bhh_in[:])
            bout_sb = rp.tile([1, VS], BF, tag="bout")
            nc.gpsimd.dma_start(bout_sb[:], bout_in[:])

            # big out-projection stream: one 1MB DMA per matvec tile
            wt_tiles = []
            for n in range(NT):
                wtn = wtp.tile([128, KH * 512], BF, tag="wt")
                nc.sync.dma_start(wtn[:], wout_in[n, :, :])
                wt_tiles.append(wtn)

            # ---- attention scores: concat(e,h) @ attn_W.T + attn_b -------
            psA = pp.tile([1, L], F32, tag="chain")
            for j in range(16):
                lhsT = ecol[:, j:j + 1] if j < KH else hcol[:, j - KH:j - KH + 1]
                nc.tensor.matmul(psA[:], lhsT, wa_sb[:, j * L:(j + 1) * L],
                                 start=(j == 0), stop=(j == 15))
            la_sb = rp.tile([1, L], F32, tag="la")
            nc.vector.tensor_tensor(la_sb[:], psA[:], ab_sb[:], ALU.add)
            nmA = rp.tile([1, 1], F32, tag="nmA")
            nc.vector.reduce_max(nmA[:], la_sb[:], axis=AX.X, negate=True)
            expA = rp.tile([1, L], F32, tag="expA")
            sA = rp.tile([1, 1], F32, tag="sA")
            nc.scalar.activation(expA[:], la_sb[:], ACTF.Exp,
                                 bias=nmA[:], accum_out=sA[:])
            rA = rp.tile([1, 1], F32, tag="rA")
            nc.vector.reciprocal(rA[:], sA[:])
            aw_f = rp.tile([1, L], F32, tag="aw_f")
            nc.vector.tensor_scalar_mul(aw_f[:], expA[:], rA[:])
            nc.scalar.dma_start(out_attn[:], aw_f[:])
            aw_bf = rp.tile([1, L], BF, tag="aw_bf")
            nc.vector.tensor_scalar_mul(aw_bf[:], expA[:], rA[:])
            aw_col = cp.tile([128, 4], BF, tag="aw_col")
            for k in range(4):
                nc.gpsimd.dma_start(aw_col[:, k:k + 1],
                                    aw_bf[0:1, k * 128:(k + 1) * 128])

            # ---- attn_applied = attn_w @ enc -----------------------------
            psB0 = pp.tile([1, 512], F32, tag="chain")
            psB1 = pp.tile([1, 512], F32, tag="chain")
            for c in range(4):
                nc.tensor.matmul(psB0[:], aw_col[:, c:c + 1],
                                 enc_sb[:, c * H:c * H + 512],
                                 start=(c == 0), stop=(c == 3))
            for c in range(4):
                nc.tensor.matmul(psB1[:], aw_col[:, c:c + 1],
                                 enc_sb[:, c * H + 512:(c + 1) * H],
                                 start=(c == 0), stop=(c == 3))
            aa_bf = rp.tile([1, H], BF, tag="aa_bf")
            nc.scalar.copy(aa_bf[0:1, 0:512], psB0[:])
            nc.scalar.copy(aa_bf[0:1, 512:1024], psB1[:])
            aa_col = cp.tile([128, KH], BF, tag="aa_col")
            for k in range(KH):
                nc.gpsimd.dma_start(aa_col[:, k:k + 1],
                                    aa_bf[0:1, k * 128:(k + 1) * 128])

            # ---- x = relu(concat(e, attn_applied) @ comb_W.T + comb_b) ---
            psC0 = pp.tile([1, 512], F32, tag="chain")
            psC1 = pp.tile([1, 512], F32, tag="chain")
            for j in range(16):
                lhsT = ecol[:, j:j + 1] if j < KH else aa_col[:, j - KH:j - KH + 1]
                nc.tensor.matmul(psC0[:], lhsT, comb_sb[:, j * H:j * H + 512],
                                 start=(j == 0), stop=(j == 15))
            for j in range(16):
                lhsT = ecol[:, j:j + 1] if j < KH else aa_col[:, j - KH:j - KH + 1]
                nc.tensor.matmul(psC1[:], lhsT, comb_sb[:, j * H + 512:(j + 1) * H],
                                 start=(j == 0), stop=(j == 15))
            x_row = rp.tile([1, H], F32, tag="x_row")
            nc.vector.tensor_tensor(x_row[0:1, 0:512], psC0[:],
                                    cb_sb[0:1, 0:512], ALU.add)
            nc.vector.tensor_tensor(x_row[0:1, 512:1024], psC1[:],
                                    cb_sb[0:1, 512:1024], ALU.add)
            xr = rp.tile([1, H], F32, tag="xr")
            nc.scalar.activation(xr[:], x_row[:], ACTF.Relu)
            x_col = cp.tile([128, KH], BF, tag="x_col")
            for k in range(KH):
                nc.gpsimd.dma_start(x_col[:, k:k + 1],
                                    xr[0:1, k * 128:(k + 1) * 128])

            # ---- GRU step (this core's 128 dims of each gate) ------------
            psGX = pp.tile([1, 384], F32, tag="chain")
            psGH = pp.tile([1, 384], F32, tag="chain")
            for j in range(KH):
                nc.tensor.matmul(psGX[:], x_col[:, j:j + 1],
                                 wih_sb[:, j * 384:(j + 1) * 384],
                                 start=(j == 0), stop=(j == KH - 1))
            for j in range(KH):
                nc.tensor.matmul(psGH[:], hcol[:, j:j + 1],
                                 whh_sb[:, j * 384:(j + 1) * 384],
                                 start=(j == 0), stop=(j == KH - 1))
            gx = rp.tile([1, 384], F32, tag="gx")
            nc.vector.tensor_tensor(gx[:], psGX[:], bih_sb[:], ALU.add)
            gh = rp.tile([1, 384], F32, tag="gh")
            nc.vector.tensor_tensor(gh[:], psGH[:], bhh_sb[:], ALU.add)
            prerz = rp.tile([1, 256], F32, tag="prerz")
            nc.vector.tensor_tensor(prerz[:], gx[0:1, 0:256],
                                    gh[0:1, 0:256], ALU.add)
            rz = rp.tile([1, 256], F32, tag="rz")
            nc.scalar.activation(rz[:], prerz[:], ACTF.Sigmoid)
            tmpn = rp.tile([1, 128], F32, tag="tmpn")
            nc.vector.tensor_tensor(tmpn[:], rz[0:1, 0:128],
                                    gh[0:1, 256:384], ALU.mult)
            npre = rp.tile([1, 128], F32, tag="npre")
            nc.vector.tensor_tensor(npre[:], gx[0:1, 256:384], tmpn[:], ALU.add)
            nt_ = rp.tile([1, 128], F32, tag="nt")
            nc.scalar.activation(nt_[:], npre[:], ACTF.Tanh)
            dts = rp.tile([1, 128], F32, tag="dts")
            nc.vector.tensor_tensor(dts[:], hsl[:], nt_[:], ALU.subtract)
            zd = rp.tile([1, 128], F32, tag="zd")
            nc.vector.tensor_tensor(zd[:], rz[0:1, 128:256], dts[:], ALU.mult)
            hnc = rp.tile([1, 128], F32, tag="hnc")
            nc.vector.tensor_tensor(hnc[:], nt_[:], zd[:], ALU.add)

            # ---- AllGather h_new slices ----------------------------------
            hb_in = dp.tile([1, 128], F32, tag="h_in")
            hb_out = dp.tile([8, 128], F32, tag="h_out")
            nc.gpsimd.dma_start(hb_in[:], hnc[:])
            nc.gpsimd.collective_compute(
                "AllGather", ALU.bypass, replica_groups=RG,
                ins=[hb_in.opt()], outs=[hb_out.opt()])
            hrow = rp.tile([1, H], F32, tag="hrow")
            nc.gpsimd.dma_start(hrow[:], hb_out[:])
            nc.scalar.dma_start(out_h[:], hrow[:])
            hncol = cp.tile([128, KH], BF, tag="hncol")
            nc.gpsimd.dma_start(hncol[:], hb_out[:].rearrange("k p -> p k"))

            # ---- out projection + per-tile softmax stats -----------------
            logits = rp.tile([1, VS], F32, tag="logits")
            nmax = rp.tile([1, NT], F32, tag="nmax")
            ssum = rp.tile([1, NT], F32, tag="ssum")
            for n in range(NT):
                psM = mp.tile([1, 512], F32, tag="mv")
                for k in range(KH):
                    nc.tensor.matmul(psM[:], hncol[:, k:k + 1],
                                     wt_tiles[n][:, k * 512:(k + 1) * 512],
                                     start=(k == 0), stop=(k == KH - 1))
                sl = logits[0:1, n * 512:(n + 1) * 512]
                nc.vector.tensor_tensor(sl, psM[:],
                                        bout_sb[0:1, n * 512:(n + 1) * 512],
                                        ALU.add)
                nc.vector.reduce_max(nmax[0:1, n:n + 1], sl, axis=AX.X,
                                     negate=True)
                esc = ep.tile([1, 512], F32, tag="esc")
                nc.scalar.activation(esc[:], sl, ACTF.Exp,
                                     bias=nmax[0:1, n:n + 1],
                                     accum_out=ssum[0:1, n:n + 1])

            # ---- local logsumexp stats -----------------------------------
            negM = rp.tile([1, 1], F32, tag="negM")
            nc.vector.tensor_reduce(negM[:], nmax[:], AX.X, op=ALU.min)
            e13 = rp.tile([1, NT], F32, tag="e13")
            nc.scalar.activation(e13[:], nmax[:], ACTF.Exp,
                                 bias=negM[:], scale=-1.0)
            prod = rp.tile([1, NT], F32, tag="prod")
            nc.vector.tensor_tensor(prod[:], ssum[:], e13[:], ALU.mult)
            sloc = rp.tile([1, 1], F32, tag="sloc")
            nc.vector.reduce_sum(sloc[:], prod[:], axis=AX.X)

            stats = rp.tile([1, 2], F32, tag="stats")
            nc.vector.tensor_scalar_mul(stats[0:1, 0:1], negM[:], -1.0)
            nc.vector.tensor_copy(stats[0:1, 1:2], sloc[:])

            # ---- AllGather stats, global logZ ----------------------------
            sb_in = dp.tile([1, 2], F32, tag="s_in")
            sb_out = dp.tile([8, 2], F32, tag="s_out")
            nc.gpsimd.dma_start(sb_in[:], stats[:])
            nc.gpsimd.collective_compute(
                "AllGather", ALU.bypass, replica_groups=RG,
                ins=[sb_in.opt()], outs=[sb_out.opt()])
            sall = rp.tile([1, 16], F32, tag="sall")
            nc.gpsimd.dma_start(sall[:], sb_out[:])

            negMg = rp.tile([1, 1], F32, tag="negMg")
            nc.vector.reduce_max(negMg[:], sall[0:1, 0:16:2], axis=AX.X,
                                 negate=True)
            e8 = rp.tile([1, 8], F32, tag="e8")
            nc.scalar.activation(e8[:], sall[0:1, 0:16:2], ACTF.Exp,
                                 bias=negMg[:])
            prod8 = rp.tile([1, 8], F32, tag="prod8")
            nc.vector.tensor_tensor(prod8[:], sall[0:1, 1:16:2], e8[:],
                                    ALU.mult)
            sg = rp.tile([1, 1], F32, tag="sg")
            nc.vector.reduce_sum(sg[:], prod8[:], axis=AX.X)
            lns = rp.tile([1, 1], F32, tag="lns")
            nc.scalar.activation(lns[:], sg[:], ACTF.Ln)
            neglz = rp.tile([1, 1], F32, tag="neglz")
            nc.vector.tensor_tensor(neglz[:], negMg[:], lns[:], ALU.subtract)

            # ---- logp = logits - logZ ------------------------------------
            nc.scalar.activation(logits[:], logits[:], ACTF.Identity,
                                 bias=neglz[:])
            nc.scalar.dma_start(out_logp[:], logits[:])

    nc.compile()
    return nc


def _prep_in_maps(input, hidden, encoder_outputs, emb, attn_W, attn_b,
                  comb_W, comb_b, gru_w_ih, gru_w_hh, gru_b_ih, gru_b_hh,
                  out_W, out_b):
    f32 = np.float32
    idx = int(np.asarray(input).reshape(-1)[0])
    emb = np.asarray(emb, dtype=f32)
    e_row = np.ascontiguousarray(emb[idx])
    h_row = np.asarray(hidden, dtype=f32).reshape(H)
    enc = np.asarray(encoder_outputs, dtype=f32)
    attn_W = np.asarray(attn_W, dtype=f32)
    attn_b = np.asarray(attn_b, dtype=f32)
    comb_W = np.asarray(comb_W, dtype=f32)
    comb_b = np.asarray(comb_b, dtype=f32)
    gru_w_ih = np.asarray(gru_w_ih, dtype=f32)
    gru_w_hh = np.asarray(gru_w_hh, dtype=f32)
    gru_b_ih = np.asarray(gru_b_ih, dtype=f32)
    gru_b_hh = np.asarray(gru_b_hh, dtype=f32)
    out_W = np.asarray(out_W, dtype=f32)
    out_b = np.asarray(out_b, dtype=f32)

    ecol = np.ascontiguousarray(e_row.reshape(KH, 128).T).astype(BF16)
    hcol = np.ascontiguousarray(h_row.reshape(KH, 128).T).astype(BF16)

    wa = np.ascontiguousarray(
        attn_W.T.reshape(16, 128, L).transpose(1, 0, 2).reshape(128, 16 * L)
    ).astype(BF16)
    enc_sb = np.ascontiguousarray(
        enc.reshape(4, 128, H).transpose(1, 0, 2).reshape(128, 4 * H)
    ).astype(BF16)
    comb = np.ascontiguousarray(
        comb_W.T.reshape(16, 128, H).transpose(1, 0, 2).reshape(128, 16 * H)
    ).astype(BF16)
    ab = attn_b.reshape(1, L)
    cb = comb_b.reshape(1, H)

    wpad = np.zeros((VP, H), dtype=f32)
    wpad[:V] = out_W
    bpad = np.full((VP,), -1e30, dtype=f32)
    bpad[:V] = out_b

    in_maps = []
    for c in range(NCORES):
        rows = np.concatenate([np.arange(g * H + c * 128, g * H + (c + 1) * 128)
                               for g in range(3)])
        wih = np.ascontiguousarray(
            gru_w_ih[rows].T.reshape(KH, 128, 384).transpose(1, 0, 2)
            .reshape(128, KH * 384)).astype(BF16)
        whh = np.ascontiguousarray(
            gru_w_hh[rows].T.reshape(KH, 128, 384).transpose(1, 0, 2)
            .reshape(128, KH * 384)).astype(BF16)
        bih = gru_b_ih[rows].reshape(1, 384)
        bhh = gru_b_hh[rows].reshape(1, 384)
        hsl = h_row[c * 128:(c + 1) * 128].reshape(1, 128)

        wt = wpad[c * VS:(c + 1) * VS]            # (VS, H)
        wout = np.ascontiguousarray(
            wt.T.reshape(KH, 128, NT, 512).transpose(2, 1, 0, 3)
            .reshape(NT, 128, KH * 512)).astype(BF16)
        bout = bpad[c * VS:(c + 1) * VS].reshape(1, VS)

        in_maps.append({
            "ecol": ecol, "hcol": hcol, "hsl": np.ascontiguousarray(hsl),
            "wa": wa, "enc": enc_sb, "comb": comb,
            "wih": wih, "whh": whh,
            "bih": np.ascontiguousarray(bih), "bhh": np.ascontiguousarray(bhh),
            "ab": np.ascontiguousarray(ab), "cb": np.ascontiguousarray(cb),
            "wout": wout, "bout": np.ascontiguousarray(bout).astype(BF16),
        })
    return in_maps


def kernel(**inputs):
    global _CACHED, LAST_RESULT
    if _CACHED is None:
        _CACHED = _build()
    nc = _CACHED
    in_maps = _prep_in_maps(**inputs)
    res = bass_utils.run_bass_kernel_spmd(
        nc, in_maps, core_ids=list(range(NCORES)))
    LAST_RESULT = res

    logp_parts = [res.results[c]["out_logp"][0] for c in range(NCORES)]
    logp = np.concatenate(logp_parts)[:V].reshape(1, V).astype(np.float32)
    h_new = res.results[0]["out_h"].reshape(1, 1, H).astype(np.float32)
    attn_w = res.results[0]["out_attn"].reshape(1, L).astype(np.float32)
    return logp, h_new, attn_w


# revision 13
# speedup vs baseline: 1.1408x; 1.1408x over previous
"""Trainium2 Bass kernel for a single-step attention GRU decoder (AttnDecoderRNN).

Computation (see reference):
    embedded = emb[input]                                  (1, H)
    attn_w   = softmax(concat(e, h) @ attn_W.T + attn_b)   (1, L)
    attn_app = attn_w @ encoder_outputs                    (1, H)
    x        = relu(concat(e, attn_app) @ comb_W.T + comb_b)
    GRU step -> h_new                                      (1, H)
    logp     = log_softmax(h_new @ out_W.T + out_b)        (1, V)

Distribution over 8 NeuronCores:
  - out_W / out_b sharded over the (padded) vocab dim, bf16 weights.
  - GRU weights sharded over the gate-output dim (128 rows of each of r/z/n
    per core); h_new slices AllGathered.
  - attention / combine weights replicated (bf16); encoder outputs replicated.
  - log_softmax via per-core (max, sumexp) stats + tiny AllGather.

Host does only layout work: embedding-row gather, shard slicing, transposes,
dtype casts, padding, final unshard/concat.
"""

import os
import sys

for _p in ("/opt/trn_rl_repo",):
    if _p not in sys.path and os.path.isdir(_p):
        sys.path.insert(0, _p)

import numpy as np
import ml_dtypes

import concourse.bass as bass
import concourse.bacc as bacc
import concourse.tile as tile
import concourse.mybir as mybir
import concourse.bass_utils as bass_utils

BF16 = ml_dtypes.bfloat16
F32 = mybir.dt.float32
BF = mybir.dt.bfloat16
AX = mybir.AxisListType
ALU = mybir.AluOpType
ACTF = mybir.ActivationFunctionType

H = 1024
L = 512
V = 50257
NCORES = 8
KH = H // 128          # 8 h-chunks
VS = 6656              # per-core padded vocab shard (13 * 512)
VP = NCORES * VS       # 53248 padded vocab
NT = VS // 512         # 13 matvec tiles per core
RG = [list(range(NCORES))]

LAST_RESULT = None     # BassKernelResults of the most recent run (for test.py)

_CACHED = None


def _build():
    """Build + compile the SPMD graph (identical on all 8 cores)."""
    nc = bacc.Bacc("TRN2", target_bir_lowering=False, debug=False,
                   num_devices=NCORES)

    # ---- DRAM I/O -------------------------------------------------------
    ecol_in = nc.dram_tensor("ecol", [128, KH], BF, kind="ExternalInput")
    hcol_in = nc.dram_tensor("hcol", [128, KH], BF, kind="ExternalInput")
    hsl_in = nc.dram_tensor("hsl", [1, 128], F32, kind="ExternalInput")
    wa_in = nc.dram_tensor("wa", [128, 16 * L], BF, kind="ExternalInput")
    enc_in = nc.dram_tensor("enc", [128, 4 * H], BF, kind="ExternalInput")
    comb_in = nc.dram_tensor("comb", [128, 16 * H], BF, kind="ExternalInput")
    wih_in = nc.dram_tensor("wih", [128, KH * 384], BF, kind="ExternalInput")
    whh_in = nc.dram_tensor("whh", [128, KH * 384], BF, kind="ExternalInput")
    bih_in = nc.dram_tensor("bih", [1, 384], F32, kind="ExternalInput")
    bhh_in = nc.dram_tensor("bhh", [1, 384], F32, kind="ExternalInput")
    ab_in = nc.dram_tensor("ab", [1, L], F32, kind="ExternalInput")
    cb_in = nc.dram_tensor("cb", [1, H], F32, kind="ExternalInput")
    wout_in = nc.dram_tensor("wout", [NT, 128, KH * 512], BF, kind="ExternalInput")
    bout_in = nc.dram_tensor("bout", [1, VS], BF, kind="ExternalInput")

    out_logp = nc.dram_tensor("out_logp", [1, VS], F32, kind="ExternalOutput")
    out_h = nc.dram_tensor("out_h", [1, H], F32, kind="ExternalOutput")
    out_attn = nc.dram_tensor("out_attn", [1, L], F32, kind="ExternalOutput")

    with tile.TileContext(nc) as tc:
        with (
            tc.tile_pool(name="wpool", bufs=1) as wp,
            tc.tile_pool(name="wtpool", bufs=3) as wtp,
            tc.tile_pool(name="rows", bufs=1) as rp,
            tc.tile_pool(name="esc", bufs=2) as ep,
            tc.tile_pool(name="cols", bufs=1) as cp,
            tc.tile_pool(name="chain_ps", bufs=4, space="PSUM") as pp,
            tc.tile_pool(name="mv_ps", bufs=3, space="PSUM") as mp,
            tc.tile_pool(name="dram", bufs=1, space="DRAM") as dp,
        ):
            # ---- warm-up collective (absorbs ncfw init; fire-and-forget,
            # nothing reads its output so the gpsimd queue never blocks on it)
            warm = rp.tile([1, 64], F32, tag="warm")
            nc.gpsimd.memset(warm[:], 0.0)
            win_b = dp.tile([1, 64], F32, tag="warm_in")
            wout_b = dp.tile([8, 64], F32, tag="warm_out")
            nc.gpsimd.dma_start(win_b[:], warm[:])
            nc.gpsimd.collective_compute(
                "AllGather", ALU.bypass, replica_groups=RG,
                ins=[win_b.opt()], outs=[wout_b.opt()])

            # ---- weight / vector loads -----------------------------------
            wa_sb = wp.tile([128, 16 * L], BF, tag="wa")
            nc.sync.dma_start(wa_sb[:], wa_in[:])
            enc_sb = wp.tile([128, 4 * H], BF, tag="enc")
            nc.sync.dma_start(enc_sb[:], enc_in[:])
            comb_sb = wp.tile([128, 16 * H], BF, tag="comb")
            nc.sync.dma_start(comb_sb[:], comb_in[:])
            wih_sb = wp.tile([128, KH * 384], BF, tag="wih")
            nc.sync.dma_start(wih_sb[:], wih_in[:])
            whh_sb = wp.tile([128, KH * 384], BF, tag="whh")
            nc.sync.dma_start(whh_sb[:], whh_in[:])

            ecol = cp.tile([128, KH], BF, tag="ecol")
            nc.gpsimd.dma_start(ecol[:], ecol_in[:])
            hcol = cp.tile([128, KH], BF, tag="hcol")
            nc.gpsimd.dma_start(hcol[:], hcol_in[:])
            hsl = rp.tile([1, 128], F32, tag="hsl")
            nc.gpsimd.dma_start(hsl[:], hsl_in[:])
            ab_sb = rp.tile([1, L], F32, tag="ab")
            nc.gpsimd.dma_start(ab_sb[:], ab_in[:])
            cb_sb = rp.tile([1, H], F32, tag="cb")
            nc.gpsimd.dma_start(cb_sb[:], cb_in[:])
            bih_sb = rp.tile([1, 384], F32, tag="bih")
            nc.gpsimd.dma_start(bih_sb[:], bih_in[:])
            bhh_sb = rp.tile([1, 384], F32, tag="bhh")
            nc.gpsimd.dma_start(bhh_sb[:], bhh_in[:])
            bout_sb = rp.tile([1, VS], BF, tag="bout")
            nc.gpsimd.dma_start(bout_sb[:], bout_in[:])

            # big out-projection stream: one 1MB DMA per matvec tile
            wt_tiles = []
            for n in range(NT):
                wtn = wtp.tile([128, KH * 512], BF, tag="wt")
                nc.sync.dma_start(wtn[:], wout_in[n, :, :])
                wt_tiles.append(wtn)

            # ---- attention scores: concat(e,h) @ attn_W.T + attn_b -------
            psA = pp.tile([1, L], F32, tag="chain")
            for j in range(16):
                lhsT = ecol[:, j:j + 1] if j < KH else hcol[:, j - KH:j - KH + 1]
                nc.tensor.matmul(psA[:], lhsT, wa_sb[:, j * L:(j + 1) * L],
                                 start=(j == 0), stop=(j == 15))
            la_sb = rp.tile([1, L], F32, tag="la")
            nc.vector.tensor_tensor(la_sb[:], psA[:], ab_sb[:], ALU.add)
            nmA = rp.tile([1, 1], F32, tag="nmA")
            nc.vector.reduce_max(nmA[:], la_sb[:], axis=AX.X, negate=True)
            expA = rp.tile([1, L], F32, tag="expA")
            sA = rp.tile([1, 1], F32, tag="sA")
            nc.scalar.activation(expA[:], la_sb[:], ACTF.Exp,
                                 bias=nmA[:], accum_out=sA[:])
            rA = rp.tile([1, 1], F32, tag="rA")
            nc.vector.reciprocal(rA[:], sA[:])
            aw_f = rp.tile([1, L], F32, tag="aw_f")
            nc.vector.tensor_scalar_mul(aw_f[:], expA[:], rA[:])
            nc.scalar.dma_start(out_attn[:], aw_f[:])
            aw_bf = rp.tile([1, L], BF, tag="aw_bf")
            nc.vector.tensor_scalar_mul(aw_bf[:], expA[:], rA[:])
            aw_col = cp.tile([128, 4], BF, tag="aw_col")
            for k in range(4):
                nc.gpsimd.dma_start(aw_col[:, k:k + 1],
                                    aw_bf[0:1, k * 128:(k + 1) * 128])

            # ---- attn_applied = attn_w @ enc -----------------------------
            psB0 = pp.tile([1, 512], F32, tag="chain")
            psB1 = pp.tile([1, 512], F32, tag="chain")
            for c in range(4):
                nc.tensor.matmul(psB0[:], aw_col[:, c:c + 1],
                                 enc_sb[:, c * H:c * H + 512],
                                 start=(c == 0), stop=(c == 3))
            for c in range(4):
                nc.tensor.matmul(psB1[:], aw_col[:, c:c + 1],
                                 enc_sb[:, c * H + 512:(c + 1) * H],
                                 start=(c == 0), stop=(c == 3))
            aa_bf = rp.tile([1, H], BF, tag="aa_bf")
            nc.scalar.copy(aa_bf[0:1, 0:512], psB0[:])
            nc.scalar.copy(aa_bf[0:1, 512:1024], psB1[:])
            aa_col = cp.tile([128, KH], BF, tag="aa_col")
            for k in range(KH):
                nc.gpsimd.dma_start(aa_col[:, k:k + 1],
                                    aa_bf[0:1, k * 128:(k + 1) * 128])

            # ---- x = relu(concat(e, attn_applied) @ comb_W.T + comb_b) ---
            psC0 = pp.tile([1, 512], F32, tag="chain")
            psC1 = pp.tile([1, 512], F32, tag="chain")
            for j in range(16):
                lhsT = ecol[:, j:j + 1] if j < KH else aa_col[:, j - KH:j - KH + 1]
                nc.tensor.matmul(psC0[:], lhsT, comb_sb[:, j * H:j * H + 512],
                                 start=(j == 0), stop=(j == 15))
            for j in range(16):
                lhsT = ecol[:, j:j + 1] if j < KH else aa_col[:, j - KH:j - KH + 1]
                nc.tensor.matmul(psC1[:], lhsT, comb_sb[:, j * H + 512:(j + 1) * H],
                                 start=(j == 0), stop=(j == 15))
            x_row = rp.tile([1, H], F32, tag="x_row")
            nc.vector.tensor_tensor(x_row[0:1, 0:512], psC0[:],
                                    cb_sb[0:1, 0:512], ALU.add)
            nc.vector.tensor_tensor(x_row[0:1, 512:1024], psC1[:],
                                    cb_sb[0:1, 512:1024], ALU.add)
            xr = rp.tile([1, H], F32, tag="xr")
            nc.scalar.activation(xr[:], x_row[:], ACTF.Relu)
            x_col = cp.tile([128, KH], BF, tag="x_col")
            for k in range(KH):
                nc.gpsimd.dma_start(x_col[:, k:k + 1],
                                    xr[0:1, k * 128:(k + 1) * 128])

            # ---- GRU step (this core's 128 dims of each gate) ------------
            psGX = pp.tile([1, 384], F32, tag="chain")
            psGH = pp.tile([1, 384], F32, tag="chain")
            for j in range(KH):
                nc.tensor.matmul(psGX[:], x_col[:, j:j + 1],
                                 wih_sb[:, j * 384:(j + 1) * 384],
                                 start=(j == 0), stop=(j == KH - 1))
            for j in range(KH):
                nc.tensor.matmul(psGH[:], hcol[:, j:j + 1],
                                 whh_sb[:, j * 384:(j + 1) * 384],
                                 start=(j == 0), stop=(j == KH - 1))
            gx = rp.tile([1, 384], F32, tag="gx")
            nc.vector.tensor_tensor(gx[:], psGX[:], bih_sb[:], ALU.add)
            gh = rp.tile([1, 384], F32, tag="gh")
            nc.vector.tensor_tensor(gh[:], psGH[:], bhh_sb[:], ALU.add)
            prerz = rp.tile([1, 256], F32, tag="prerz")
            nc.vector.tensor_tensor(prerz[:], gx[0:1, 0:256],
                                    gh[0:1, 0:256], ALU.add)
            rz = rp.tile([1, 256], F32, tag="rz")
            nc.scalar.activation(rz[:], prerz[:], ACTF.Sigmoid)
            tmpn = rp.tile([1, 128], F32, tag="tmpn")
            nc.vector.tensor_tensor(tmpn[:], rz[0:1, 0:128],
                                    gh[0:1, 256:384], ALU.mult)
            npre = rp.tile([1, 128], F32, tag="npre")
            nc.vector.tensor_tensor(npre[:], gx[0:1, 256:384], tmpn[:], ALU.add)
            nt_ = rp.tile([1, 128], F32, tag="nt")
            nc.scalar.activation(nt_[:], npre[:], ACTF.Tanh)
            dts = rp.tile([1, 128], F32, tag="dts")
            nc.vector.tensor_tensor(dts[:], hsl[:], nt_[:], ALU.subtract)
            zd = rp.tile([1, 128], F32, tag="zd")
            nc.vector.tensor_tensor(zd[:], rz[0:1, 128:256], dts[:], ALU.mult)
            hnc = rp.tile([1, 128], F32, tag="hnc")
            nc.vector.tensor_tensor(hnc[:], nt_[:], zd[:], ALU.add)

            # ---- AllGather h_new slices ----------------------------------
            hb_in = dp.tile([1, 128], F32, tag="h_in")
            hb_out = dp.tile([8, 128], F32, tag="h_out")
            nc.gpsimd.dma_start(hb_in[:], hnc[:])
            nc.gpsimd.collective_compute(
                "AllGather", ALU.bypass, replica_groups=RG,
                ins=[hb_in.opt()], outs=[hb_out.opt()])
            hrow = rp.tile([1, H], F32, tag="hrow")
            nc.gpsimd.dma_start(hrow[:], hb_out[:])
            nc.scalar.dma_start(out_h[:], hrow[:])
            hncol = cp.tile([128, KH], BF, tag="hncol")
            nc.gpsimd.dma_start(hncol[:], hb_out[:].rearrange("k p -> p k"))

            # ---- out projection + per-tile softmax stats -----------------
            logits = rp.tile([1, VS], F32, tag="logits")
            nmax = rp.tile([1, NT], F32, tag="nmax")
            ssum = rp.tile([1, NT], F32, tag="ssum")
            for n in range(NT):
                psM = mp.tile([1, 512], F32, tag="mv")
                for k in range(KH):
                    nc.tensor.matmul(psM[:], hncol[:, k:k + 1],
                                     wt_tiles[n][:, k * 512:(k + 1) * 512],
                                     start=(k == 0), stop=(k == KH - 1))
                sl = logits[0:1, n * 512:(n + 1) * 512]
                nc.vector.tensor_tensor(sl, psM[:],
                                        bout_sb[0:1, n * 512:(n + 1) * 512],
                                        ALU.add)
                nc.vector.reduce_max(nmax[0:1, n:n + 1], sl, axis=AX.X,
                                     negate=True)
                esc = ep.tile([1, 512], F32, tag="esc")
                nc.scalar.activation(esc[:], sl, ACTF.Exp,
                                     bias=nmax[0:1, n:n + 1],
                                     accum_out=ssum[0:1, n:n + 1])

            # ---- local logsumexp stats -----------------------------------
            negM = rp.tile([1, 1], F32, tag="negM")
            nc.vector.tensor_reduce(negM[:], nmax[:], AX.X, op=ALU.min)
            e13 = rp.tile([1, NT], F32, tag="e13")
            nc.scalar.activation(e13[:], nmax[:], ACTF.Exp,
                                 bias=negM[:], scale=-1.0)
            prod = rp.tile([1, NT], F32, tag="prod")
            nc.vector.tensor_tensor(prod[:], ssum[:], e13[:], ALU.mult)
            sloc = rp.tile([1, 1], F32, tag="sloc")
            nc.vector.reduce_sum(sloc[:], prod[:], axis=AX.X)

            stats = rp.tile([1, 2], F32, tag="stats")
            nc.vector.tensor_scalar_mul(stats[0:1, 0:1], negM[:], -1.0)
            nc.vector.tensor_copy(stats[0:1, 1:2], sloc[:])

            # ---- AllGather stats, global logZ ----------------------------
            sb_in = dp.tile([1, 2], F32, tag="s_in")
            sb_out = dp.tile([8, 2], F32, tag="s_out")
            nc.gpsimd.dma_start(sb_in[:], stats[:])
            nc.gpsimd.collective_compute(
                "AllGather", ALU.bypass, replica_groups=RG,
                ins=[sb_in.opt()], outs=[sb_out.opt()])
            sall = rp.tile([1, 16], F32, tag="sall")
            nc.gpsimd.dma_start(sall[:], sb_out[:])

            negMg = rp.tile([1, 1], F32, tag="negMg")
            nc.vector.reduce_max(negMg[:], sall[0:1, 0:16:2], axis=AX.X,
                                 negate=True)
            e8 = rp.tile([1, 8], F32, tag="e8")
            nc.scalar.activation(e8[:], sall[0:1, 0:16:2], ACTF.Exp,
                                 bias=negMg[:])
            prod8 = rp.tile([1, 8], F32, tag="prod8")
            nc.vector.tensor_tensor(prod8[:], sall[0:1, 1:16:2], e8[:],
                                    ALU.mult)
            sg = rp.tile([1, 1], F32, tag="sg")
            nc.vector.reduce_sum(sg[:], prod8[:], axis=AX.X)
            lns = rp.tile([1, 1], F32, tag="lns")
            nc.scalar.activation(lns[:], sg[:], ACTF.Ln)
            neglz = rp.tile([1, 1], F32, tag="neglz")
            nc.vector.tensor_tensor(neglz[:], negMg[:], lns[:], ALU.subtract)

            # ---- logp = logits - logZ ------------------------------------
            nc.scalar.activation(logits[:], logits[:], ACTF.Identity,
                                 bias=neglz[:])
            nc.scalar.dma_start(out_logp[:], logits[:])

    nc.compile()
    return nc


def _prep_in_maps(input, hidden, encoder_outputs, emb, attn_W, attn_b,
                  comb_W, comb_b, gru_w_ih, gru_w_hh, gru_b_ih, gru_b_hh,
                  out_W, out_b):
    f32 = np.float32
    idx = int(np.asarray(input).reshape(-1)[0])
    emb = np.asarray(emb, dtype=f32)
    e_row = np.ascontiguousarray(emb[idx])
    h_row = np.asarray(hidden, dtype=f32).reshape(H)
    enc = np.asarray(encoder_outputs, dtype=f32)
    attn_W = np.asarray(attn_W, dtype=f32)
    attn_b = np.asarray(attn_b, dtype=f32)
    comb_W = np.asarray(comb_W, dtype=f32)
    comb_b = np.asarray(comb_b, dtype=f32)
    gru_w_ih = np.asarray(gru_w_ih, dtype=f32)
    gru_w_hh = np.asarray(gru_w_hh, dtype=f32)
    gru_b_ih = np.asarray(gru_b_ih, dtype=f32)
    gru_b_hh = np.asarray(gru_b_hh, dtype=f32)
    out_W = np.asarray(out_W, dtype=f32)
    out_b = np.asarray(out_b, dtype=f32)

    ecol = np.ascontiguousarray(e_row.reshape(KH, 128).T).astype(BF16)
    hcol = np.ascontiguousarray(h_row.reshape(KH, 128).T).astype(BF16)

    wa = np.ascontiguousarray(
        attn_W.T.reshape(16, 128, L).transpose(1, 0, 2).reshape(128, 16 * L)
    ).astype(BF16)
    enc_sb = np.ascontiguousarray(
        enc.reshape(4, 128, H).transpose(1, 0, 2).reshape(128, 4 * H)
    ).astype(BF16)
    comb = np.ascontiguousarray(
        comb_W.T.reshape(16, 128, H).transpose(1, 0, 2).reshape(128, 16 * H)
    ).astype(BF16)
    ab = attn_b.reshape(1, L)
    cb = comb_b.reshape(1, H)

    wpad = np.zeros((VP, H), dtype=f32)
    wpad[:V] = out_W
    bpad = np.full((VP,), -1e30, dtype=f32)
    bpad[:V] = out_b

    in_maps = []
    for c in range(NCORES):
        rows = np.concatenate([np.arange(g * H + c * 128, g * H + (c + 1) * 128)
                               for g in range(3)])
        wih = np.ascontiguousarray(
            gru_w_ih[rows].T.reshape(KH, 128, 384).transpose(1, 0, 2)
            .reshape(128, KH * 384)).astype(BF16)
        whh = np.ascontiguousarray(
            gru_w_hh[rows].T.reshape(KH, 128, 384).transpose(1, 0, 2)
            .reshape(128, KH * 384)).astype(BF16)
        bih = gru_b_ih[rows].reshape(1, 384)
        bhh = gru_b_hh[rows].reshape(1, 384)
        hsl = h_row[c * 128:(c + 1) * 128].reshape(1, 128)

        wt = wpad[c * VS:(c + 1) * VS]            # (VS, H)
        wout = np.ascontiguousarray(
            wt.T.reshape(KH, 128, NT, 512).transpose(2, 1, 0, 3)
            .reshape(NT, 128, KH * 512)).astype(BF16)
        bout = bpad[c * VS:(c + 1) * VS].reshape(1, VS)

        in_maps.append({
            "ecol": ecol, "hcol": hcol, "hsl": np.ascontiguousarray(hsl),
            "wa": wa, "enc": enc_sb, "comb": comb,
            "wih": wih, "whh": whh,
            "bih": np.ascontiguousarray(bih), "bhh": np.ascontiguousarray(bhh),
            "ab": np.ascontiguousarray(ab), "cb": np.ascontiguousarray(cb),
            "wout": wout, "bout": np.ascontiguousarray(bout).astype(BF16),
        })
    return in_maps


def kernel(**inputs):
    global _CACHED, LAST_RESULT
    if _CACHED is None:
        _CACHED = _build()
    nc = _CACHED
    in_maps = _prep_in_maps(**inputs)
    res = bass_utils.run_bass_kernel_spmd(
        nc, in_maps, core_ids=list(range(NCORES)))
    LAST_RESULT = res

    logp_parts = [res.results[c]["out_logp"][0] for c in range(NCORES)]
    logp = np.concatenate(logp_parts)[:V].reshape(1, V).astype(np.float32)
    h_new = res.results[0]["out_h"].reshape(1, 1, H).astype(np.float32)
    attn_w = res.results[0]["out_attn"].reshape(1, L).astype(np.float32)
    return logp, h_new, attn_w
